# revision 12
# baseline (speedup 1.0000x reference)
"""Trainium2 Bass kernel for nn_LutLinear (BCQ/LUT-quantized linear layer).

Math (K=4096, N=4096, WBIT=3, GROUP=128, APOT=3):
  bits[k, b, n]  = bit (k%32) of binaryWeight[k//32, b, n]
  B              = 2*bits - 1                        (in {-1, +1})
  scale[n, b, g] = sum_a 2^alpha[n, b, g, a]
  out[n] = sum_{g,b} scale[n,b,g] * (sum_{k in group g} x[k] * B[k,b,n]) + bias[n]

Strategy (tensor-parallel over N, 8 cores, N'=512 each), raw bass:
  * No startup barrier / sem-clear: input DMAs issue at t=0; semaphore +
    DMA-state cleanup runs at END of program (gpsimd, after out-DMA), so
    re-runs still see a clean slate.
  * bw shard DMA'd in two column halves (n-halves) so bit-unpack overlaps
    the second half's transfer.
  * DVE bit-unpack: (w16 << t) & 0x4040 on int16 lanes -> fp8e4 planes
    (byte 0x40 = 2.0).  16 ops (8 shifts x 2 halves).
  * PE: per (half, shift, c, bit): matmul lhsT = block-diagonal x bank
    (bf16) [128, 32], rhs = fp8 plane view [128, 256] (stride 4), out =
    psum [32, 256].  3 psum col-strips (one per weight bit) run
    concurrently; fp32 warm-up matmuls flip the HAM clock gate early.
  * Tail: pr[q,n] = psum96 * scale (bf16, DVE, split in n-halves),
    ones^T @ pr on PE per half (row 96 = bias2 = bias - sum scale*S_g),
    Act copies psum->SBUF per half, single out DMA.
"""

import os
import sys

for _p in ("/opt/trn_rl_repo", "/opt/pypackages"):
    if os.path.isdir(_p) and _p not in sys.path:
        sys.path.insert(0, _p)

from contextlib import ExitStack

import ml_dtypes
import numpy as np

import concourse.bass as bass
from concourse import bacc, mybir
from concourse.bass_utils import run_bass_kernel_spmd

K = 4096
N = 4096
GROUP = 128
WBIT = 3
G = K // GROUP          # 32 groups
NCORES = 8
NS = N // NCORES        # 512 output features per core
NH = NS // 2            # 256 per half
WORDS = K // 32         # 128 packed words per (b, n)
Q = WBIT * G            # 96 psum rows
BF16 = ml_dtypes.bfloat16

# bank layout: [w, j(32), g(32)] bf16 -> 2048 B/partition
BANK_B = 32 * 32 * 2              # 2048
ONES_OFF = BANK_B                  # bf16 ones at bytes 2048:2050
CSB_OFF = BANK_B + 4               # bf16 csb at bytes 2052:3076 (aligned)
AUX_B = CSB_OFF + NS * 2           # 3076

_CACHE = {}


def _build(nc):
    f32 = mybir.dt.float32
    i32 = mybir.dt.int32
    i16 = mybir.dt.int16
    i8 = mybir.dt.int8
    bf16 = mybir.dt.bfloat16
    f8 = mybir.dt.float8e4
    LSL = mybir.AluOpType.logical_shift_left
    LSR = mybir.AluOpType.logical_shift_right
    AND = mybir.AluOpType.bitwise_and

    bwd = nc.dram_tensor("bwd", [WORDS, 2 * WBIT * NH], i32, kind="ExternalInput")
    auxd = nc.dram_tensor("auxd", [128, AUX_B], i8, kind="ExternalInput")
    b2d = nc.dram_tensor("b2d", [1, NS], bf16, kind="ExternalInput")
    out = nc.dram_tensor("out", [1, NS], f32, kind="ExternalOutput")

    ctx = ExitStack()
    wsb = ctx.enter_context(nc.sbuf_tensor("wsb", [WORDS, 2 * WBIT * NH], i32))
    m16 = ctx.enter_context(nc.sbuf_tensor("m16", [128, 16 * 1536], i16))
    aux = ctx.enter_context(nc.sbuf_tensor("aux", [128, AUX_B], i8))
    pr = ctx.enter_context(nc.sbuf_tensor("pr", [Q + 1, NS], bf16))
    outsb = ctx.enter_context(nc.sbuf_tensor("outsb", [1, NS], f32))
    warm = ctx.enter_context(nc.sbuf_tensor("warm", [128, NS], f32))
    ps96 = ctx.enter_context(nc.psum_tensor("ps96", [Q, NS], f32))
    psO = ctx.enter_context(nc.psum_tensor("psO", [1, NS], f32))

    s_bw0 = ctx.enter_context(nc.semaphore("s_bw0"))    # bw half 0 DMA done
    s_bw1 = ctx.enter_context(nc.semaphore("s_bw1"))    # bw half 1 DMA done
    s_aux = ctx.enter_context(nc.semaphore("s_aux"))    # aux image DMA done
    s_b2 = ctx.enter_context(nc.semaphore("s_b2"))      # bias2 row DMA done
    s_up = ctx.enter_context(nc.semaphore("s_up"))      # DVE unpack ops
    s_mm = ctx.enter_context(nc.semaphore("s_mm"))      # psum strip stops
    s_pr = ctx.enter_context(nc.semaphore("s_pr"))      # pr halves done
    s_red = ctx.enter_context(nc.semaphore("s_red"))    # psO halves done
    s_out = ctx.enter_context(nc.semaphore("s_out"))    # outsb halves done
    s_done = ctx.enter_context(nc.semaphore("s_done"))  # out DMA done
    s_pool = ctx.enter_context(nc.semaphore("s_pool"))  # warm memset done

    sem_nums = sorted(
        s.num
        for s in (s_bw0, s_bw1, s_aux, s_b2, s_up, s_mm, s_pr, s_red, s_out, s_done, s_pool)
    )

    w16 = wsb[:].bitcast(i16)  # [128, 6144]; half h = cols 3072h:3072h+3072

    def plane(h, s):
        c0 = 1536 * (8 * h + s)
        return m16[:, c0 : c0 + 1536]

    # fp8 view of a plane: [p, b, c, n] (bytes laid out b-major, n, then c)
    def plane_f8(h, s):
        return plane(h, s).bitcast(f8).rearrange(
            "p (b n c) -> p b c n", b=WBIT, n=NH, c=4
        )

    xv = aux[:, 0:BANK_B].bitcast(bf16).rearrange(
        "p (j g) -> p j g", j=G, g=G
    )
    ones = aux[0 : Q + 1, ONES_OFF : ONES_OFF + 2].bitcast(bf16)  # [97, 1]
    csb = aux[0:Q, CSB_OFF : CSB_OFF + NS * 2].bitcast(bf16)      # [96, 512]

    def unpack(eng, h, s):
        dst = plane(h, s)
        src = w16[:, 1536 * h : 1536 * (h + 1)]
        if s < 7:
            return eng.tensor_scalar(dst, src, 6 - s, 0x4040, LSL, AND)
        return eng.tensor_scalar(dst, src, 1, 0x4040, LSR, AND)

    with nc.Block(no_gpsimd_drain=True) as block:

        @block.sync
        def _(sync):
            half = WBIT * NH  # 768 i32 cols per half
            sync.dma_start(wsb[:, 0:half], bwd[:, 0:half]).then_inc(s_bw0, 16)
            sync.dma_start(wsb[:, half : 2 * half], bwd[:, half : 2 * half]).then_inc(
                s_bw1, 16
            )
            sync.dma_start(aux[:], auxd[:, :]).then_inc(s_aux, 16)
            sync.dma_start(pr[Q : Q + 1, :], b2d[0:1, :]).then_inc(s_b2, 16)
            sync.wait_ge(s_out, 2)
            sync.dma_start(out[0:1, :], outsb[:]).then_inc(s_done, 16)
            sync.wait_ge(s_done, 16)

        @block.vector
        def _(vector):
            for h, s_bwh in ((0, s_bw0), (1, s_bw1)):
                vector.wait_ge(s_bwh, 16)
                for s in range(8):
                    unpack(vector, h, s).then_inc(s_up, 1)
            # pr half 0 (cols 0:256): needs the 3 half-0 strip stops + csb
            vector.wait_ge(s_mm, 3)
            vector.wait_ge(s_aux, 16)
            vector.tensor_tensor(
                pr[0:Q, 0:NH], ps96[:, 0:NH], csb[:, 0:NH],
                mybir.AluOpType.mult,
            ).then_inc(s_pr, 1)
            # pr half 1 (cols 256:512): needs all 6 strip stops
            vector.wait_ge(s_mm, 6)
            vector.tensor_tensor(
                pr[0:Q, NH:NS], ps96[:, NH:NS], csb[:, NH:NS],
                mybir.AluOpType.mult,
            ).then_inc(s_pr, 1)

        @block.gpsimd
        def _(gpsimd):
            gpsimd.memset(warm[:], 0.0).then_inc(s_pool, 1)
            # End-of-run cleanup so a re-execution starts from a clean slate.
            gpsimd.wait_ge(s_done, 16)
            for rng in _compact_ranges(sem_nums):
                gpsimd.dma_reset(rng)
                gpsimd.sem_clear(rng)

        @block.tensor
        def _(tensor):
            # fp32 warm-up matmuls keep the PE busy so the HAM clock gate
            # flips to 2.4 GHz before the real matmuls start.
            tensor.wait_ge(s_pool, 1)
            for _ in range(3):
                tensor.matmul(
                    psO[0:1, :], warm[:, 0:1], warm[:, :], start=True, stop=True
                )
            tensor.wait_ge(s_aux, 16)
            for h in range(2):
                nsl = slice(NH * h, NH * h + NH)
                for s in range(8):
                    tensor.wait_ge(s_up, 8 * h + s + 1)
                    mp = plane_f8(h, s)
                    for c in range(4):
                        j = 8 * c + s
                        for b in range(WBIT):
                            mm = tensor.matmul(
                                ps96[32 * b : 32 * b + 32, nsl],
                                xv[:, j, :],
                                mp[:, b, c, :],
                                start=(s == 0 and c == 0),
                                stop=(s == 7 and c == 3),
                                skip_group_check=True,
                            )
                            if s == 7 and c == 3:
                                mm.then_inc(s_mm, 1)
                # reduce for this half once pr half is ready (+ bias2 row)
                tensor.wait_ge(s_pr, h + 1)
                tensor.wait_ge(s_b2, 16)
                tensor.matmul(
                    psO[0:1, nsl], ones[:, :], pr[:, nsl],
                    start=True, stop=True, skip_group_check=True,
                ).then_inc(s_red, 1)

        @block.scalar
        def _(scalar):
            for h in range(2):
                nsl = slice(NH * h, NH * h + NH)
                scalar.wait_ge(s_red, h + 1)
                scalar.copy(outsb[0:1, nsl], psO[0:1, nsl]).then_inc(s_out, 1)

    ctx.close()


def _compact_ranges(nums):
    out = []
    start = prev = nums[0]
    for n in nums[1:]:
        if n == prev + 1:
            prev = n
            continue
        out.append(range(start, prev + 1))
        start = prev = n
    out.append(range(start, prev + 1))
    return out


def _get_nc():
    if "nc" not in _CACHE:
        nc = bacc.Bacc(
            "TRN2",
            target_bir_lowering=False,
            debug=False,
            enable_asserts=False,
            num_devices=1,
        )
        _build(nc)
        nc.compile()
        _CACHE["nc"] = nc
    return _CACHE["nc"]


def _prep_inputs(x, binaryWeight, alpha, bias):
    """Host-side shard + layout/encoding prep."""
    x = np.asarray(x, dtype=np.float32).reshape(K)
    binaryWeight = np.asarray(binaryWeight, dtype=np.int32)
    alpha = np.asarray(alpha, dtype=np.int32)
    bias = np.asarray(bias, dtype=np.float32).reshape(N)

    # Block-diagonal lhsT bank: xall[w, j*32 + g] = x[32w + j] iff g == w//4
    xall = np.zeros((WORDS, G, G), dtype=np.float32)  # [w, j, g]
    w = np.arange(WORDS)
    for j in range(G):
        xall[w, j, w // 4] = x[32 * w + j]
    xallb = xall.reshape(WORDS, G * G).astype(BF16)
    xt = xallb.astype(np.float64)
    sg = xt.reshape(WORDS, G, G).sum(axis=(0, 1))  # effective group sums [G]
    bank = xallb.view(np.uint8).reshape(WORDS, BANK_B)

    # scale[n, b, g] = sum_a 2^alpha (exact in bf16)
    scale = np.exp2(alpha.astype(np.float64)).sum(axis=-1)  # [N, WBIT, G]

    onesv = np.zeros(128, dtype=BF16)
    onesv[: Q + 1] = BF16(1.0)

    in_maps = []
    for cc in range(NCORES):
        nsl = slice(cc * NS, (cc + 1) * NS)
        # bw columns reordered to (h, b, n256)
        bw_sh = (
            np.ascontiguousarray(
                binaryWeight[:, :, nsl]
                .reshape(WORDS, WBIT, 2, NH)
                .transpose(0, 2, 1, 3)
            ).reshape(WORDS, 2 * WBIT * NH)
        )
        sc = scale[nsl]  # [NS, WBIT, G]
        consts = np.zeros((128, NS), dtype=BF16)
        for b in range(WBIT):
            consts[32 * b : 32 * b + 32, :] = sc[:, b, :].T.astype(BF16)
        b2 = (bias[nsl] - np.einsum("nbg,g->n", sc, sg)).astype(BF16)

        aux = np.zeros((128, AUX_B), dtype=np.uint8)
        aux[:, 0:BANK_B] = bank
        aux[:, ONES_OFF : ONES_OFF + 2] = onesv.view(np.uint8).reshape(128, 2)
        aux[:, CSB_OFF:AUX_B] = consts.view(np.uint8)
        in_maps.append(
            {
                "bwd": bw_sh,
                "auxd": aux.view(np.int8),
                "b2d": b2.reshape(1, NS),
            }
        )
    return in_maps


def _run(inputs, trace=False, **kw):
    nc = _get_nc()
    in_maps = _prep_inputs(**inputs)
    res = run_bass_kernel_spmd(
        nc, in_maps, core_ids=list(range(NCORES)), trace=trace, **kw
    )
    outs = [res.results[cc]["out"].reshape(NS) for cc in range(NCORES)]
    full = np.concatenate(outs).reshape(1, N).astype(np.float32)
    return full, res


def kernel(**inputs):
    out, _ = _run(inputs, trace=False)
    return out


# revision 23
# speedup vs baseline: 1.0408x; 1.0408x over previous
"""Trainium2 Bass kernel for nn_LutLinear (BCQ/LUT-quantized linear layer).

Math (K=4096, N=4096, WBIT=3, GROUP=128, APOT=3):
  bits[k, b, n]  = bit (k%32) of binaryWeight[k//32, b, n]
  B              = 2*bits - 1                        (in {-1, +1})
  scale[n, b, g] = sum_a 2^alpha[n, b, g, a]
  out[n] = sum_{g,b} scale[n,b,g] * (sum_{k in group g} x[k] * B[k,b,n]) + bias[n]

Strategy (tensor-parallel over N, 8 cores, N'=512 each), raw bass (no Tile
framework -- manual semaphores, so the epilogue semaphore-clear churn that
dominated the Tile version's teardown disappears):
  * DVE bit-unpack: (words << t) & 0x4040 on int16 lanes yields fp8e4 planes
    (byte 0x40 = 2.0), 8 whole-tile ops.
  * PE: 96 matmuls lhsT = block-diagonal x bank (bf16) [128, 32], rhs = fp8
    bit-plane view [128, 512] (stride 4).  The 3 b-matmuls per (s, c) target
    psum col-blocks 0/32/64 and column-tile 3-way on the array.
  * Tail: prod[q, n'] = psum96 * scale (bf16, one DVE op), ones^T @ prod on
    PE (97th row = bias2 = bias - sum_q scale*S_g), DVE copy psum->SBUF, DMA.
"""

import os
import sys

for _p in ("/opt/trn_rl_repo", "/opt/pypackages"):
    if os.path.isdir(_p) and _p not in sys.path:
        sys.path.insert(0, _p)

from contextlib import ExitStack

import ml_dtypes
import numpy as np

import concourse.bass as bass
from concourse import bacc, mybir
from concourse.bass_utils import run_bass_kernel_spmd

K = 4096
N = 4096
GROUP = 128
WBIT = 3
G = K // GROUP          # 32 groups
NCORES = 8
NS = N // NCORES        # 512 output features per core
WORDS = K // 32         # 128 packed words per (b, n)
Q = WBIT * G            # 96 psum rows
BF16 = ml_dtypes.bfloat16

_CACHE = {}


def _build(nc):
    f32 = mybir.dt.float32
    i32 = mybir.dt.int32
    i16 = mybir.dt.int16
    bf16 = mybir.dt.bfloat16
    f8 = mybir.dt.float8e4
    LSL = mybir.AluOpType.logical_shift_left
    LSR = mybir.AluOpType.logical_shift_right
    AND = mybir.AluOpType.bitwise_and

    bw = nc.dram_tensor("bw", [WORDS, WBIT * NS], i32, kind="ExternalInput")
    xall = nc.dram_tensor("xall", [WORDS, G * G], bf16, kind="ExternalInput")
    consts = nc.dram_tensor("consts", [Q, NS], bf16, kind="ExternalInput")
    bias2 = nc.dram_tensor("bias2", [1, NS], bf16, kind="ExternalInput")
    out = nc.dram_tensor("out", [1, NS], f32, kind="ExternalOutput")

    ctx = ExitStack()
    wsb = ctx.enter_context(nc.sbuf_tensor("wsb", [WORDS, WBIT * NS], i32))
    m16 = ctx.enter_context(nc.sbuf_tensor("m16", [128, 8 * 3072], i16))
    xsb = ctx.enter_context(nc.sbuf_tensor("xsb", [WORDS, G * G], bf16))
    csb = ctx.enter_context(nc.sbuf_tensor("csb", [Q, NS], bf16))
    pr = ctx.enter_context(nc.sbuf_tensor("pr", [Q + 1, NS], bf16))
    warm = ctx.enter_context(nc.sbuf_tensor("warm", [128, 1024], bf16))
    ones = ctx.enter_context(nc.sbuf_tensor("ones", [Q + 1, 1], bf16))
    outsb = ctx.enter_context(nc.sbuf_tensor("outsb", [1, NS], f32))
    ps96 = ctx.enter_context(nc.psum_tensor("ps96", [Q, NS], f32))
    psO = ctx.enter_context(nc.psum_tensor("psO", [1, NS], f32))

    s_bw = ctx.enter_context(nc.semaphore("s_bw"))
    s_b2 = ctx.enter_context(nc.semaphore("s_b2"))
    s_x = ctx.enter_context(nc.semaphore("s_x"))
    s_cs = ctx.enter_context(nc.semaphore("s_cs"))
    s_pool = ctx.enter_context(nc.semaphore("s_pool"))
    s_up = ctx.enter_context(nc.semaphore("s_up"))
    s_mm = ctx.enter_context(nc.semaphore("s_mm"))
    s_pr = ctx.enter_context(nc.semaphore("s_pr"))
    s_red = ctx.enter_context(nc.semaphore("s_red"))
    s_out = ctx.enter_context(nc.semaphore("s_out"))
    s_done = ctx.enter_context(nc.semaphore("s_done"))

    # Re-run safety: clear kernel semaphores before any engine proceeds.
    sem_nums = sorted(
        s.num
        for s in (s_bw, s_b2, s_x, s_cs, s_pool, s_up, s_mm, s_pr, s_red, s_out, s_done)
    )
    for rng in _compact_ranges(sem_nums):
        nc.gpsimd.dma_reset(rng)
        nc.gpsimd.sem_clear(rng)
    nc._nrt_pseudo_barrier()

    w16 = wsb[:].bitcast(i16)                       # [128, 3072]
    xv = xsb[:].rearrange("p (j g) -> p j g", j=G)  # [128, 32, 32]

    with nc.Block(no_gpsimd_drain=True) as block:

        @block.sync
        def _(sync):
            sync.dma_start(wsb[:], bw[:, :]).then_inc(s_bw, 16)
            sync.dma_start(pr[Q : Q + 1, :], bias2[0:1, :]).then_inc(s_b2, 16)
            sync.wait_ge(s_out, 1)
            sync.dma_start(out[0:1, :], outsb[:]).then_inc(s_done, 16)
            sync.wait_ge(s_done, 16)

        @block.scalar
        def _(scalar):
            scalar.dma_start(xsb[:], xall[:, :]).then_inc(s_x, 16)
            scalar.dma_start(csb[:], consts[:, :]).then_inc(s_cs, 16)

        @block.gpsimd
        def _(gpsimd):
            gpsimd.memset(warm[:], 0.0).then_inc(s_pool, 1)
            gpsimd.memset(ones[:], 1.0).then_inc(s_pool, 1)

        @block.vector
        def _(vector):
            vector.wait_ge(s_bw, 16)
            for s in range(8):
                dst = m16[:, 3072 * s : 3072 * (s + 1)]
                if s < 7:
                    vector.tensor_scalar(dst, w16, 6 - s, 0x4040, LSL, AND).then_inc(
                        s_up, 1
                    )
                else:
                    vector.tensor_scalar(dst, w16, 1, 0x4040, LSR, AND).then_inc(
                        s_up, 1
                    )
            vector.wait_ge(s_mm, WBIT)
            vector.wait_ge(s_cs, 16)
            vector.tensor_tensor(
                pr[0:Q, :], ps96[:], csb[:], mybir.AluOpType.mult
            ).then_inc(s_pr, 1)
            vector.wait_ge(s_red, 1)
            vector.tensor_scalar(
                outsb[:], psO[:], 0.0, None, mybir.AluOpType.add
            ).then_inc(s_out, 1)

        @block.tensor
        def _(tensor):
            tensor.wait_ge(s_pool, 1)
            wf32 = warm[:].bitcast(f32)             # [128, 512]
            tensor.matmul(
                psO[0:1, 0:512], wf32[:, 0:1], wf32[:, :], start=True, stop=True
            )
            tensor.matmul(
                psO[0:1, 0:512], wf32[:, 0:1], wf32[:, :], start=True, stop=True
            )
            tensor.matmul(
                psO[0:1, 0:512], warm[:, 0:1], warm[:, 0:512], start=True, stop=True
            )
            tensor.wait_ge(s_x, 16)
            for s in range(8):
                mv = m16[:, 3072 * s : 3072 * (s + 1)].bitcast(f8)
                mv = mv.rearrange("p (b n c) -> p b c n", b=WBIT, n=NS, c=4)
                tensor.wait_ge(s_up, s + 1)
                for c in range(4):
                    j = 8 * c + s
                    for b in range(WBIT):
                        mm = tensor.matmul(
                            ps96[32 * b : 32 * b + 32, :],
                            xv[:, j, :],
                            mv[:, b, c, :],
                            start=(s == 0 and c == 0),
                            stop=(s == 7 and c == 3),
                            skip_group_check=True,
                        )
                        if s == 7 and c == 3:
                            mm.then_inc(s_mm, 1)
            tensor.wait_ge(s_pr, 1)
            tensor.wait_ge(s_b2, 16)
            tensor.wait_ge(s_pool, 2)
            tensor.matmul(
                psO[0:1, :], ones[:, :], pr[:, :], start=True, stop=True
            ).then_inc(s_red, 1)

    ctx.close()


def _compact_ranges(nums):
    out = []
    start = prev = nums[0]
    for n in nums[1:]:
        if n == prev + 1:
            prev = n
            continue
        out.append(range(start, prev + 1))
        start = prev = n
    out.append(range(start, prev + 1))
    return out


def _get_nc():
    if "nc" not in _CACHE:
        nc = bacc.Bacc(
            "TRN2",
            target_bir_lowering=False,
            debug=False,
            enable_asserts=False,
            num_devices=1,
        )
        _build(nc)
        nc.compile()
        _CACHE["nc"] = nc
    return _CACHE["nc"]


def _prep_inputs(x, binaryWeight, alpha, bias):
    """Host-side shard + layout/encoding prep."""
    x = np.asarray(x, dtype=np.float32).reshape(K)
    binaryWeight = np.asarray(binaryWeight, dtype=np.int32)
    alpha = np.asarray(alpha, dtype=np.int32)
    bias = np.asarray(bias, dtype=np.float32).reshape(N)

    # Block-diagonal lhsT bank: xall[w, j*32 + g] = x[32w + j] iff g == w//4
    xall = np.zeros((WORDS, G, G), dtype=np.float32)  # [w, j, g]
    w = np.arange(WORDS)
    for j in range(G):
        xall[w, j, w // 4] = x[32 * w + j]
    xallb = xall.reshape(WORDS, G * G).astype(BF16)

    xb = xallb.astype(np.float32)
    sg = xb.reshape(WORDS, G, G).sum(axis=(0, 1))  # effective group sums [G]

    # scale[n, b, g] = sum_a 2^alpha (exact in bf16)
    scale = np.exp2(alpha.astype(np.float32)).sum(axis=-1)  # [N, WBIT, G]

    in_maps = []
    for cc in range(NCORES):
        nsl = slice(cc * NS, (cc + 1) * NS)
        bw_sh = np.ascontiguousarray(binaryWeight[:, :, nsl]).reshape(
            WORDS, WBIT * NS
        )
        sc = scale[nsl]  # [NS, WBIT, G]
        consts = np.zeros((Q, NS), dtype=np.float32)
        for b in range(WBIT):
            consts[32 * b : 32 * b + 32, :] = sc[:, b, :].T
        b2 = bias[nsl] - np.einsum("nbg,g->n", sc, sg)
        in_maps.append(
            {
                "bw": bw_sh,
                "xall": xallb,
                "consts": consts.astype(BF16),
                "bias2": b2.reshape(1, NS).astype(BF16),
            }
        )
    return in_maps


def _run(inputs, trace=False, **kw):
    nc = _get_nc()
    in_maps = _prep_inputs(**inputs)
    res = run_bass_kernel_spmd(
        nc, in_maps, core_ids=list(range(NCORES)), trace=trace, **kw
    )
    outs = [res.results[cc]["out"].reshape(NS) for cc in range(NCORES)]
    full = np.concatenate(outs).reshape(1, N).astype(np.float32)
    return full, res


def kernel(**inputs):
    out, _ = _run(inputs, trace=False)
    return out

